# revision 35
# baseline (speedup 1.0000x reference)
"""GAT self-attention Trainium2 kernel (v2, bf16 data path).

Full inputs -> shard graphs over 8 NeuronCores -> full output.

Math (per graph n, reference reformulated):
  g_i = sigmoid(relu(q @ W1_i) @ W2_i)            [2d]
  u_i^L = W_i @ (g_i[:d] * a_i[:d])               [k]   (left projector)
  u_i^R = W_i @ (g_i[d:] * a_i[d:])               [k]   (right projector)
  l_i = X @ u_i^L ; r_i = X @ u_i^R               [E]
  S[i,j] = lrelu(l_t[i] + r_t[j]), t = adj[i,j]
  E' = exp(S) * (adj > 0); rs = rowsum(E')
  h = X @ W_2 ; hs = h / rs[:, None]
  out = E'^T @ hs          (== softmax(scores)^T @ (X @ W_2))

Key implementation points:
  - everything bf16 except the score rank-2 matmuls (f32r) and PSUM.
  - adj shipped as bf16 so type masks are DVE tensor_scalar is_equal in
    4x mode; no gpsimd is_equal, no int32 adj DMA.
  - one DMA per tensor (HWDGE slot costs ~630ns per DMA instruction).
  - scores: per type a single rank-2 matmul from a persistent 12-row
    stack [1s, l_t, r_t, 1s] built by one stt pass from the LR matmul.
  - type select via 2 copy_predicated; adj==0 handled by multiplying
    exp by (adj>0) in the same stt pass that row-sums E'.
  - softmax normalization folded into h's PSUM->SBUF copy (scale by
    1/rs), so no extra pass over the [E,E] matrix.
"""
import numpy as np
from contextlib import ExitStack

import concourse.bass as bass
import concourse.tile as tile
from concourse import mybir, bacc
from concourse.masks import make_identity

F32 = mybir.dt.float32
F32R = mybir.dt.float32r
BF16 = mybir.dt.bfloat16
U8 = mybir.dt.uint8
AF = mybir.ActivationFunctionType
OP = mybir.AluOpType

N_CORES = 8
N, E, K, D = 64, 512, 512, 512   # graphs, entities, in_dim, out_dim
NG = N // N_CORES                # graphs per core
NT = 3                           # edge types
P = 128
EC = E // P                      # 4 partition chunks of E
KC = K // P
DC2 = (2 * D) // P               # 8 chunks of the 2d gate dim


def build(nc, reps=1):
    x = nc.dram_tensor("x", [NG, E, K], BF16, kind="ExternalInput").ap()
    adjf = nc.dram_tensor("adjf", [NG, E, E], BF16, kind="ExternalInput").ap()
    qv = nc.dram_tensor("qv", [NG, K], F32, kind="ExternalInput").ap()
    Wt = nc.dram_tensor("Wt", [NT, K, D], BF16, kind="ExternalInput").ap()
    at = nc.dram_tensor("at", [NT, 2 * D], F32, kind="ExternalInput").ap()
    W1 = nc.dram_tensor("W1", [NT, K, 2 * D], BF16, kind="ExternalInput").ap()
    W2q = nc.dram_tensor("W2q", [NT, 2 * D, 2 * D], BF16, kind="ExternalInput").ap()
    out = nc.dram_tensor("out", [NG, E, D], BF16, kind="ExternalOutput").ap()
    nc._gat_io = (x, adjf, qv, Wt, at, W1, W2q, out)
    _build_once(nc, reps)


def _build_once(nc, reps=1):
    x, adjf, qv, Wt, at, W1, W2q, out = nc._gat_io
    with tile.TileContext(nc) as tc, ExitStack() as ctx:
        # ---------------- persistent tiles ----------------
        pers = ctx.enter_context(tc.tile_pool(name="pers", bufs=1))
        identb = pers.tile([P, P], BF16)
        make_identity(nc, identb[:])
        identf = pers.tile([P, P], F32)
        make_identity(nc, identf[:])
        # U6[k%128, kc, c, n]: c in 0..2 -> left type c, 3..5 -> right
        U6 = pers.tile([P, KC, 2 * NT, NG], BF16)
        Wt2_sb = pers.tile([P, KC, D], BF16)
        nc.scalar.dma_start(Wt2_sb[:], Wt[2].rearrange("(c p) d -> p c d", p=P))
        aT = pers.tile([P, DC2, NT], F32)
        # Score-stack tiles: operand pair for type t at base partition 32*t
        # (compute engines may only touch partition ranges based at 0/32/64/96,
        # so data rows are scattered by DMA, ones rows by legal memset or a
        # one-time DMA).  lhsT_t = stkL[32t:32t+2] = [1s; l_t];
        # rhs_t = stkR[32t:32t+2] = [r_t; 1s].  A/B buffering across graphs.
        ones3 = pers.tile([NT, E], F32)
        nc.vector.memset(ones3[:], 1.0)
        stkL = [pers.tile([66, E], F32, name=f"stkL{i}") for i in range(2)]
        stkR = [pers.tile([66, E], F32, name=f"stkR{i}") for i in range(2)]
        for sb in range(2):
            for t in range(NT):
                nc.vector.memset(stkL[sb][32 * t:32 * t + 1, :], 1.0)
            nc.sync.dma_start(stkR[sb][1:66:32, :], ones3[:])

        # ---------------- PSUM pools (8 banks total) ----------------
        ps_s1 = ctx.enter_context(tc.tile_pool(name="ps_s1", bufs=2, space="PSUM"))
        ps_s23 = ctx.enter_context(tc.tile_pool(name="ps_s23", bufs=1, space="PSUM"))
        ps_h = ctx.enter_context(tc.tile_pool(name="ps_h", bufs=2, space="PSUM"))
        ps_o = ctx.enter_context(tc.tile_pool(name="ps_o", bufs=1, space="PSUM"))
        ps_lr = ctx.enter_context(tc.tile_pool(name="ps_lr", bufs=1, space="PSUM"))

        # ---------------- prep: gates + projector vectors ----------------
        def run_prep():
          with tc.tile_pool(name="prep", bufs=2) as prep, \
               tc.tile_pool(name="prepw", bufs=1) as prepw:
            qv_nat = prep.tile([NG, K], F32, tag="qn")
            nc.scalar.dma_start(qv_nat[:], qv)
            at_nat = prep.tile([NT, 2 * D], F32, tag="an")
            nc.scalar.dma_start(at_nat[:], at)
            W1s = []
            for i in range(NT):
                W1_sb = prepw.tile([P, KC, 2 * D], BF16, name=f"W1_{i}", tag=f"w1_{i}")
                nc.scalar.dma_start(W1_sb[:], W1[i].rearrange("(c p) f -> p c f", p=P))
                W1s.append(W1_sb)
            WTs = []
            for i in range(NT):
                WTi = prepw.tile([P, EC, K], BF16, name=f"WT_{i}", tag=f"wt_{i}")
                nc.sync.dma_start_transpose(WTi[:], Wt[i])
                WTs.append(WTi)
            # qT[k%128, kc, n] via PE transposes batched in one PSUM tile
            qT = prep.tile([P, KC, NG], BF16, tag="qT")
            qps = ps_s1.tile([P, E], F32, tag="s1")
            for kc in range(KC):
                nc.tensor.transpose(
                    qps[:, kc * NG:(kc + 1) * NG],
                    qv_nat[:, kc * P:(kc + 1) * P], identf[:NG, :NG])
            nc.vector.tensor_copy(
                qT[:], qps[:, 0:KC * NG].rearrange("p (k n) -> p k n", k=KC))
            # aT[d2%128, oc, t] via PE transposes batched in one PSUM tile
            aps = ps_s1.tile([P, E], F32, tag="s1")
            for oc in range(DC2):
                nc.tensor.transpose(
                    aps[:, oc * NT:(oc + 1) * NT],
                    at_nat[:, oc * P:(oc + 1) * P], identf[:NT, :NT])
            nc.vector.tensor_copy(
                aT[:], aps[:, 0:DC2 * NT].rearrange("p (c t) -> p c t", c=DC2))

            for i in range(NT):
                # rr = relu(W1_i^T q): all 8 out-chunks in one PSUM tile
                prr = ps_s23.tile([P, 2, E], F32, tag="s23")
                prrv = prr[:, 0, 0:DC2 * NG].rearrange("p (c n) -> p c n", c=DC2)
                for oc in range(DC2):
                    for kc in range(KC):
                        nc.tensor.matmul(
                            prrv[:, oc, :], W1s[i][:, kc, oc * P:(oc + 1) * P],
                            qT[:, kc, :], start=(kc == 0), stop=(kc == KC - 1))
                rrT = prep.tile([P, DC2, NG], BF16, tag="rrT")
                nc.scalar.activation(rrT[:], prrv[:], AF.Relu)
                # gv = sigmoid(W2q_i^T rr)
                W2_sb = prep.tile([P, DC2, 2 * D], BF16, tag="w2")
                nc.scalar.dma_start(
                    W2_sb[:, :, 0:D],
                    W2q[i, :, 0:D].rearrange("(c p) f -> p c f", p=P))
                nc.scalar.dma_start(
                    W2_sb[:, :, D:2 * D],
                    W2q[i, :, D:2 * D].rearrange("(c p) f -> p c f", p=P))
                pgv = ps_s23.tile([P, 2, E], F32, tag="s23")
                pgvv = pgv[:, 0, 0:DC2 * NG].rearrange("p (c n) -> p c n", c=DC2)
                for oc in range(DC2):
                    for dc in range(DC2):
                        nc.tensor.matmul(
                            pgvv[:, oc, :], W2_sb[:, dc, oc * P:(oc + 1) * P],
                            rrT[:, dc, :], start=(dc == 0), stop=(dc == DC2 - 1))
                gvT = prep.tile([P, DC2, NG], BF16, tag="gvT")
                nc.scalar.activation(gvT[:], pgvv[:], AF.Sigmoid)
                # vT = gvT * a_i (broadcast over n)
                vT = prep.tile([P, DC2, NG], BF16, tag="vT")
                nc.vector.tensor_tensor(
                    vT[:], gvT[:], aT[:, :, i:i + 1].broadcast_to((P, DC2, NG)),
                    OP.mult)
                # U columns: left at c=i, right at c=3+i
                for sde in range(2):
                    pu = ps_s1.tile([P, E], F32, tag="s1")
                    puv = pu[:, 0:KC * NG].rearrange("p (k n) -> p k n", k=KC)
                    for kc in range(KC):
                        for dc in range(EC):
                            nc.tensor.matmul(
                                puv[:, kc, :],
                                WTs[i][:, dc, kc * P:(kc + 1) * P],
                                vT[:, sde * EC + dc, :],
                                start=(dc == 0), stop=(dc == EC - 1))
                    nc.vector.tensor_copy(U6[:, :, 3 * sde + i, :], puv[:])

        # ---------------- main per-graph pipeline ----------------
        deep = ctx.enter_context(tc.tile_pool(name="deep", bufs=2))
        p_adj = ctx.enter_context(tc.tile_pool(name="p_adj", bufs=4))
        p_xt = ctx.enter_context(tc.tile_pool(name="p_xt", bufs=4))
        p_hs = ctx.enter_context(tc.tile_pool(name="p_hs", bufs=8))
        sbuf = ctx.enter_context(tc.tile_pool(name="sbuf", bufs=2))
        small = ctx.enter_context(tc.tile_pool(name="small", bufs=2))

        def load_graph(n):
            h_eng = nc.vector if n < 4 else nc.scalar
            adj_sb = p_adj.tile([P, EC, E], BF16, tag="adj")
            nc.gpsimd.dma_start(adj_sb[:], adjf[n].rearrange("(c p) j -> p c j", p=P))
            Xt_sb = p_xt.tile([P, KC, E], BF16, tag="Xt")
            nc.sync.dma_start_transpose(Xt_sb[:], x[n])
            # h = X @ W_2 (unscaled) -- only needs Xt + Wt2, runs during prep
            h_sb = p_hs.tile([P, EC, D], BF16, tag="hs")
            for ic in range(EC):
                pH = ps_h.tile([P, D], F32, tag="ph")
                for kc in range(KC):
                    nc.tensor.matmul(pH[:], Xt_sb[:, kc, ic * P:(ic + 1) * P],
                                     Wt2_sb[:, kc, :],
                                     start=(kc == 0), stop=(kc == KC - 1))
                if n < 6:
                    nc.vector.tensor_copy(h_sb[:, ic, :], pH[:])
                else:
                    nc.scalar.copy(h_sb[:, ic, :], pH[:])
            return dict(adj_sb=adj_sb, Xt_sb=Xt_sb, h_sb=h_sb)

        def compute_graph(n, st):
            adj_sb, Xt_sb, h_sb = st["adj_sb"], st["Xt_sb"], st["h_sb"]
            # ---- masks (DVE 4x on bf16) ----
            mz = sbuf.tile([P, EC, E], BF16, tag="mz")
            nc.vector.tensor_scalar(mz[:], adj_sb[:], 0.5, None, OP.is_gt)
            m2 = sbuf.tile([P, EC, E], U8, tag="m2")
            nc.gpsimd.tensor_scalar(m2[:], adj_sb[:], 2.0, None, OP.is_equal)
            m3 = sbuf.tile([P, EC, E], U8, tag="m3")
            nc.gpsimd.tensor_scalar(m3[:], adj_sb[:], 3.0, None, OP.is_equal)

            # ---- LR rows -> score stacks ----
            pLR = ps_lr.tile([2 * NT, E], F32, tag="lr")
            for kc in range(KC):
                nc.tensor.matmul(pLR[:], U6[:, kc, :, n], Xt_sb[:, kc, :],
                                 start=(kc == 0), stop=(kc == KC - 1))
            sL, sR = stkL[n % 2], stkR[n % 2]
            stg = small.tile([2 * NT, E], F32, tag="stg")
            nc.vector.tensor_copy(stg[:], pLR[:])
            # l_t -> row 32t+1 of stkL; r_t -> row 32t of stkR (DMA scatter)
            nc.scalar.dma_start(sL[1:66:32, :], stg[0:NT, :])
            nc.scalar.dma_start(sR[0:65:32, :], stg[NT:2 * NT, :])

            # ---- per-chunk scores ----
            rs = small.tile([P, EC], F32, tag="rs")
            rsr = small.tile([P, EC], F32, tag="rsr")
            E_sb = deep.tile([P, EC, E], BF16, tag="E")
            for ic in range(EC):
                pv1 = ps_s1.tile([P, E], F32, tag="s1")
                nc.tensor.matmul(
                    pv1[:], sL[0:2, ic * P:(ic + 1) * P].bitcast(F32R),
                    sR[0:2, :].bitcast(F32R), start=True, stop=True)
                pv23 = ps_s23.tile([P, 2, E], F32, tag="s23")
                nc.tensor.matmul(
                    pv23[:, 0, :], sL[32:34, ic * P:(ic + 1) * P].bitcast(F32R),
                    sR[32:34, :].bitcast(F32R), start=True, stop=True)
                nc.tensor.matmul(
                    pv23[:, 1, :], sL[64:66, ic * P:(ic + 1) * P].bitcast(F32R),
                    sR[64:66, :].bitcast(F32R), start=True, stop=True)
                nc.vector.copy_predicated(pv1[:], m2[:, ic, :], pv23[:, 0, :])
                nc.vector.copy_predicated(pv1[:], m3[:, ic, :], pv23[:, 1, :])
                lr_sb = small.tile([P, E], BF16, tag="lrl")
                if ic % 2 == 0:
                    nc.scalar.activation(lr_sb[:], pv1[:], AF.Prelu, alpha=0.2)
                else:
                    nc.vector.scalar_tensor_tensor(lr_sb[:], pv1[:], 0.2, pv1[:],
                                                   OP.mult, OP.max)
                e1_sb = small.tile([P, E], BF16, tag="e1")
                nc.scalar.activation(e1_sb[:], lr_sb[:], AF.Exp)
                # E' = e1 * (adj>0), rowsum into rs
                nc.vector.scalar_tensor_tensor(
                    E_sb[:, ic, :], e1_sb[:], 1.0, mz[:, ic, :],
                    OP.mult, OP.mult, accum_out=rs[:, ic:ic + 1])
                nc.vector.reciprocal(rsr[:, ic:ic + 1], rs[:, ic:ic + 1])
                # softmax normalization: scale E' rows in place (DVE 4x)
                nc.vector.tensor_scalar(E_sb[:, ic, :], E_sb[:, ic, :],
                                        rsr[:, ic:ic + 1], None, OP.mult)

            # ---- out = coef^T @ h ----
            out_sb = sbuf.tile([P, EC, D], BF16, tag="osb")
            for jc in range(EC):
                pO = ps_o.tile([P, D], F32, tag="po")
                for ic in range(EC):
                    nc.tensor.matmul(pO[:], E_sb[:, ic, jc * P:(jc + 1) * P],
                                     h_sb[:, ic, :],
                                     start=(ic == 0), stop=(ic == EC - 1))
                nc.scalar.copy(out_sb[:, jc, :], pO[:])
            nc.gpsimd.dma_start(out[n].rearrange("(c p) d -> p c d", p=P), out_sb[:])

        LOOKAHEAD = 3

        def body_all(_iv=None):
            st = {}
            run_prep()
            for n in range(LOOKAHEAD):
                st[n] = load_graph(n)
            for n in range(NG):
                if n + LOOKAHEAD < NG:
                    st[n + LOOKAHEAD] = load_graph(n + LOOKAHEAD)
                compute_graph(n, st.pop(n))

        if reps == 1:
            body_all()
        else:
            with tc.For_i(0, reps, 1) as _iv:
                body_all(_iv)
    return nc


_NC_CACHE = {}
TRACE = False
_LAST = {}


def _get_nc():
    if "nc" not in _NC_CACHE:
        nc = bacc.Bacc("TRN2", target_bir_lowering=False, debug=False)
        build(nc)
        nc.compile()
        _NC_CACHE["nc"] = nc
    return _NC_CACHE["nc"]


def kernel(input_state, adj, entity_mask, query_vec, W_type, a_type,
           qattn_W1, qattn_W2):
    from concourse import bass_utils
    import ml_dtypes
    bf16 = ml_dtypes.bfloat16
    nc = _get_nc()
    x = np.ascontiguousarray(input_state).astype(bf16)
    adjf = np.ascontiguousarray(adj).astype(bf16)
    qvf = np.ascontiguousarray(query_vec, dtype=np.float32)
    Wt = np.ascontiguousarray(W_type).astype(bf16)
    at = np.ascontiguousarray(a_type, dtype=np.float32)
    W1 = np.ascontiguousarray(qattn_W1).astype(bf16)
    W2q = np.ascontiguousarray(qattn_W2).astype(bf16)

    in_maps = []
    for c in range(N_CORES):
        sl = slice(c * NG, (c + 1) * NG)
        in_maps.append({
            "x": x[sl], "adjf": adjf[sl], "qv": qvf[sl],
            "Wt": Wt, "at": at, "W1": W1, "W2q": W2q,
        })
    res = bass_utils.run_bass_kernel_spmd(nc, in_maps, core_ids=list(range(N_CORES)),
                                          trace=TRACE, stitch_traces=TRACE)
    _LAST["exec_ns"] = res.exec_time_ns
    _LAST["mean_ns"] = res.mean_exec_time_ns
    _LAST["trace"] = res.instructions_and_trace
    out = np.concatenate([r["out"] for r in res.results], axis=0)
    return out.astype(np.float32)


# revision 36
# speedup vs baseline: 1.0633x; 1.0633x over previous
"""GAT self-attention Trainium2 kernel (v2, bf16 data path).

Full inputs -> shard graphs over 8 NeuronCores -> full output.

Math (per graph n, reference reformulated):
  g_i = sigmoid(relu(q @ W1_i) @ W2_i)            [2d]
  u_i^L = W_i @ (g_i[:d] * a_i[:d])               [k]   (left projector)
  u_i^R = W_i @ (g_i[d:] * a_i[d:])               [k]   (right projector)
  l_i = X @ u_i^L ; r_i = X @ u_i^R               [E]
  S[i,j] = lrelu(l_t[i] + r_t[j]), t = adj[i,j]
  E' = exp(S) * (adj > 0); rs = rowsum(E')
  h = X @ W_2 ; hs = h / rs[:, None]
  out = E'^T @ hs          (== softmax(scores)^T @ (X @ W_2))

Key implementation points:
  - everything bf16 except the score rank-2 matmuls (f32r) and PSUM.
  - adj shipped as bf16 so type masks are DVE tensor_scalar is_equal in
    4x mode; no gpsimd is_equal, no int32 adj DMA.
  - one DMA per tensor (HWDGE slot costs ~630ns per DMA instruction).
  - scores: per type a single rank-2 matmul from a persistent 12-row
    stack [1s, l_t, r_t, 1s] built by one stt pass from the LR matmul.
  - type select via 2 copy_predicated; adj==0 handled by multiplying
    exp by (adj>0) in the same stt pass that row-sums E'.
  - softmax normalization folded into h's PSUM->SBUF copy (scale by
    1/rs), so no extra pass over the [E,E] matrix.
"""
import numpy as np
from contextlib import ExitStack

import concourse.bass as bass
import concourse.tile as tile
from concourse import mybir, bacc
from concourse.masks import make_identity

F32 = mybir.dt.float32
F32R = mybir.dt.float32r
BF16 = mybir.dt.bfloat16
U8 = mybir.dt.uint8
AF = mybir.ActivationFunctionType
OP = mybir.AluOpType

N_CORES = 8
N, E, K, D = 64, 512, 512, 512   # graphs, entities, in_dim, out_dim
NG = N // N_CORES                # graphs per core
NT = 3                           # edge types
P = 128
EC = E // P                      # 4 partition chunks of E
KC = K // P
DC2 = (2 * D) // P               # 8 chunks of the 2d gate dim


def build(nc, reps=1):
    x = nc.dram_tensor("x", [NG, E, K], BF16, kind="ExternalInput").ap()
    adjf = nc.dram_tensor("adjf", [NG, E, E], BF16, kind="ExternalInput").ap()
    qv = nc.dram_tensor("qv", [NG, K], F32, kind="ExternalInput").ap()
    Wt = nc.dram_tensor("Wt", [NT, K, D], BF16, kind="ExternalInput").ap()
    at = nc.dram_tensor("at", [NT, 2 * D], F32, kind="ExternalInput").ap()
    W1 = nc.dram_tensor("W1", [NT, K, 2 * D], BF16, kind="ExternalInput").ap()
    W2q = nc.dram_tensor("W2q", [NT, 2 * D, 2 * D], BF16, kind="ExternalInput").ap()
    out = nc.dram_tensor("out", [NG, E, D], BF16, kind="ExternalOutput").ap()
    nc._gat_io = (x, adjf, qv, Wt, at, W1, W2q, out)
    _build_once(nc, reps)


def _build_once(nc, reps=1):
    x, adjf, qv, Wt, at, W1, W2q, out = nc._gat_io
    with tile.TileContext(nc) as tc, ExitStack() as ctx:
        # ---------------- persistent tiles ----------------
        pers = ctx.enter_context(tc.tile_pool(name="pers", bufs=1))
        identb = pers.tile([P, P], BF16)
        make_identity(nc, identb[:])
        identf = pers.tile([P, P], F32)
        make_identity(nc, identf[:])
        # U6[k%128, kc, c, n]: c in 0..2 -> left type c, 3..5 -> right
        U6 = pers.tile([P, KC, 2 * NT, NG], BF16)
        Wt2_sb = pers.tile([P, KC, D], BF16)
        nc.scalar.dma_start(Wt2_sb[:], Wt[2].rearrange("(c p) d -> p c d", p=P))
        aT = pers.tile([P, DC2, NT], F32)
        # Score-stack tiles: operand pair for type t at base partition 32*t
        # (compute engines may only touch partition ranges based at 0/32/64/96,
        # so data rows are scattered by DMA, ones rows by legal memset or a
        # one-time DMA).  lhsT_t = stkL[32t:32t+2] = [1s; l_t];
        # rhs_t = stkR[32t:32t+2] = [r_t; 1s].  A/B buffering across graphs.
        ones3 = pers.tile([NT, E], F32)
        nc.vector.memset(ones3[:], 1.0)
        stkL = [pers.tile([66, E], F32, name=f"stkL{i}") for i in range(2)]
        stkR = [pers.tile([66, E], F32, name=f"stkR{i}") for i in range(2)]
        for sb in range(2):
            for t in range(NT):
                nc.vector.memset(stkL[sb][32 * t:32 * t + 1, :], 1.0)
            nc.sync.dma_start(stkR[sb][1:66:32, :], ones3[:])

        # ---------------- PSUM pools (8 banks total) ----------------
        ps_s1 = ctx.enter_context(tc.tile_pool(name="ps_s1", bufs=2, space="PSUM"))
        ps_s23 = ctx.enter_context(tc.tile_pool(name="ps_s23", bufs=1, space="PSUM"))
        ps_h = ctx.enter_context(tc.tile_pool(name="ps_h", bufs=2, space="PSUM"))
        ps_o = ctx.enter_context(tc.tile_pool(name="ps_o", bufs=1, space="PSUM"))
        ps_lr = ctx.enter_context(tc.tile_pool(name="ps_lr", bufs=1, space="PSUM"))

        # ---------------- prep: gates + projector vectors ----------------
        def run_prep():
          with tc.tile_pool(name="prep", bufs=2) as prep, \
               tc.tile_pool(name="prepw", bufs=1) as prepw:
            qv_nat = prep.tile([NG, K], F32, tag="qn")
            nc.scalar.dma_start(qv_nat[:], qv)
            at_nat = prep.tile([NT, 2 * D], F32, tag="an")
            nc.scalar.dma_start(at_nat[:], at)
            W1s = []
            for i in range(NT):
                W1_sb = prepw.tile([P, KC, 2 * D], BF16, name=f"W1_{i}", tag=f"w1_{i}")
                nc.scalar.dma_start(W1_sb[:], W1[i].rearrange("(c p) f -> p c f", p=P))
                W1s.append(W1_sb)
            WTs = []
            for i in range(NT):
                WTi = prepw.tile([P, EC, K], BF16, name=f"WT_{i}", tag=f"wt_{i}")
                nc.sync.dma_start_transpose(WTi[:], Wt[i])
                WTs.append(WTi)
            # qT[k%128, kc, n] via PE transposes batched in one PSUM tile
            qT = prep.tile([P, KC, NG], BF16, tag="qT")
            qps = ps_s1.tile([P, E], F32, tag="s1")
            for kc in range(KC):
                nc.tensor.transpose(
                    qps[:, kc * NG:(kc + 1) * NG],
                    qv_nat[:, kc * P:(kc + 1) * P], identf[:NG, :NG])
            nc.vector.tensor_copy(
                qT[:], qps[:, 0:KC * NG].rearrange("p (k n) -> p k n", k=KC))
            # aT[d2%128, oc, t] via PE transposes batched in one PSUM tile
            aps = ps_s1.tile([P, E], F32, tag="s1")
            for oc in range(DC2):
                nc.tensor.transpose(
                    aps[:, oc * NT:(oc + 1) * NT],
                    at_nat[:, oc * P:(oc + 1) * P], identf[:NT, :NT])
            nc.vector.tensor_copy(
                aT[:], aps[:, 0:DC2 * NT].rearrange("p (c t) -> p c t", c=DC2))

            for i in range(NT):
                # rr = relu(W1_i^T q): all 8 out-chunks in one PSUM tile
                prr = ps_s23.tile([P, 2, E], F32, tag="s23")
                prrv = prr[:, 0, 0:DC2 * NG].rearrange("p (c n) -> p c n", c=DC2)
                for oc in range(DC2):
                    for kc in range(KC):
                        nc.tensor.matmul(
                            prrv[:, oc, :], W1s[i][:, kc, oc * P:(oc + 1) * P],
                            qT[:, kc, :], start=(kc == 0), stop=(kc == KC - 1))
                rrT = prep.tile([P, DC2, NG], BF16, tag="rrT")
                nc.scalar.activation(rrT[:], prrv[:], AF.Relu)
                # gv = sigmoid(W2q_i^T rr)
                W2_sb = prep.tile([P, DC2, 2 * D], BF16, tag="w2")
                nc.scalar.dma_start(
                    W2_sb[:, :, 0:D],
                    W2q[i, :, 0:D].rearrange("(c p) f -> p c f", p=P))
                nc.scalar.dma_start(
                    W2_sb[:, :, D:2 * D],
                    W2q[i, :, D:2 * D].rearrange("(c p) f -> p c f", p=P))
                pgv = ps_s23.tile([P, 2, E], F32, tag="s23")
                pgvv = pgv[:, 0, 0:DC2 * NG].rearrange("p (c n) -> p c n", c=DC2)
                for oc in range(DC2):
                    for dc in range(DC2):
                        nc.tensor.matmul(
                            pgvv[:, oc, :], W2_sb[:, dc, oc * P:(oc + 1) * P],
                            rrT[:, dc, :], start=(dc == 0), stop=(dc == DC2 - 1))
                gvT = prep.tile([P, DC2, NG], BF16, tag="gvT")
                nc.scalar.activation(gvT[:], pgvv[:], AF.Sigmoid)
                # vT = gvT * a_i (broadcast over n)
                vT = prep.tile([P, DC2, NG], BF16, tag="vT")
                nc.vector.tensor_tensor(
                    vT[:], gvT[:], aT[:, :, i:i + 1].broadcast_to((P, DC2, NG)),
                    OP.mult)
                # U columns: left at c=i, right at c=3+i
                for sde in range(2):
                    pu = ps_s1.tile([P, E], F32, tag="s1")
                    puv = pu[:, 0:KC * NG].rearrange("p (k n) -> p k n", k=KC)
                    for kc in range(KC):
                        for dc in range(EC):
                            nc.tensor.matmul(
                                puv[:, kc, :],
                                WTs[i][:, dc, kc * P:(kc + 1) * P],
                                vT[:, sde * EC + dc, :],
                                start=(dc == 0), stop=(dc == EC - 1))
                    nc.vector.tensor_copy(U6[:, :, 3 * sde + i, :], puv[:])

        # ---------------- main per-graph pipeline ----------------
        deep = ctx.enter_context(tc.tile_pool(name="deep", bufs=2))
        p_adj = ctx.enter_context(tc.tile_pool(name="p_adj", bufs=4))
        p_xt = ctx.enter_context(tc.tile_pool(name="p_xt", bufs=4))
        p_hs = ctx.enter_context(tc.tile_pool(name="p_hs", bufs=8))
        sbuf = ctx.enter_context(tc.tile_pool(name="sbuf", bufs=2))
        small = ctx.enter_context(tc.tile_pool(name="small", bufs=2))

        def load_graph(n):
            h_eng = nc.vector if n < 4 else nc.scalar
            adj_sb = p_adj.tile([P, EC, E], BF16, tag="adj")
            nc.gpsimd.dma_start(adj_sb[:], adjf[n].rearrange("(c p) j -> p c j", p=P))
            Xt_sb = p_xt.tile([P, KC, E], BF16, tag="Xt")
            nc.sync.dma_start_transpose(Xt_sb[:], x[n])
            # h = X @ W_2 (unscaled) -- only needs Xt + Wt2, runs during prep
            h_sb = p_hs.tile([P, EC, D], BF16, tag="hs")
            for ic in range(EC):
                pH = ps_h.tile([P, D], F32, tag="ph")
                for kc in range(KC):
                    nc.tensor.matmul(pH[:], Xt_sb[:, kc, ic * P:(ic + 1) * P],
                                     Wt2_sb[:, kc, :],
                                     start=(kc == 0), stop=(kc == KC - 1))
                if n < 6:
                    nc.vector.tensor_copy(h_sb[:, ic, :], pH[:])
                else:
                    nc.scalar.copy(h_sb[:, ic, :], pH[:])
            return dict(adj_sb=adj_sb, Xt_sb=Xt_sb, h_sb=h_sb)

        def compute_graph(n, st):
            adj_sb, Xt_sb, h_sb = st["adj_sb"], st["Xt_sb"], st["h_sb"]
            # ---- masks (DVE 4x on bf16) ----
            mz = sbuf.tile([P, EC, E], BF16, tag="mz")
            nc.vector.tensor_scalar(mz[:], adj_sb[:], 0.5, None, OP.is_gt)
            m2 = sbuf.tile([P, EC, E], U8, tag="m2")
            nc.gpsimd.tensor_scalar(m2[:], adj_sb[:], 2.0, None, OP.is_equal)
            m3 = sbuf.tile([P, EC, E], U8, tag="m3")
            nc.gpsimd.tensor_scalar(m3[:], adj_sb[:], 3.0, None, OP.is_equal)

            # ---- LR rows -> score stacks ----
            pLR = ps_lr.tile([2 * NT, E], F32, tag="lr")
            for kc in range(KC):
                nc.tensor.matmul(pLR[:], U6[:, kc, :, n], Xt_sb[:, kc, :],
                                 start=(kc == 0), stop=(kc == KC - 1))
            sL, sR = stkL[n % 2], stkR[n % 2]
            stg = small.tile([2 * NT, E], F32, tag="stg")
            nc.vector.tensor_copy(stg[:], pLR[:])
            # l_t -> row 32t+1 of stkL; r_t -> row 32t of stkR (DMA scatter)
            nc.scalar.dma_start(sL[1:66:32, :], stg[0:NT, :])
            nc.scalar.dma_start(sR[0:65:32, :], stg[NT:2 * NT, :])

            # ---- per-chunk scores ----
            rs = small.tile([P, EC], F32, tag="rs")
            rsr = small.tile([P, EC], F32, tag="rsr")
            E_sb = deep.tile([P, EC, E], BF16, tag="E")
            for ic in range(EC):
                pv1 = ps_s1.tile([P, E], F32, tag="s1")
                nc.tensor.matmul(
                    pv1[:], sL[0:2, ic * P:(ic + 1) * P].bitcast(F32R),
                    sR[0:2, :].bitcast(F32R), start=True, stop=True)
                pv23 = ps_s23.tile([P, 2, E], F32, tag="s23")
                nc.tensor.matmul(
                    pv23[:, 0, :], sL[32:34, ic * P:(ic + 1) * P].bitcast(F32R),
                    sR[32:34, :].bitcast(F32R), start=True, stop=True)
                nc.tensor.matmul(
                    pv23[:, 1, :], sL[64:66, ic * P:(ic + 1) * P].bitcast(F32R),
                    sR[64:66, :].bitcast(F32R), start=True, stop=True)
                nc.vector.copy_predicated(pv1[:], m2[:, ic, :], pv23[:, 0, :])
                nc.vector.copy_predicated(pv1[:], m3[:, ic, :], pv23[:, 1, :])
                lr_sb = small.tile([P, E], BF16, tag="lrl")
                nc.scalar.activation(lr_sb[:], pv1[:], AF.Prelu, alpha=0.2)
                e1_sb = small.tile([P, E], BF16, tag="e1")
                nc.scalar.activation(e1_sb[:], lr_sb[:], AF.Exp)
                # E' = e1 * (adj>0), rowsum into rs
                nc.vector.scalar_tensor_tensor(
                    E_sb[:, ic, :], e1_sb[:], 1.0, mz[:, ic, :],
                    OP.mult, OP.mult, accum_out=rs[:, ic:ic + 1])
                nc.vector.reciprocal(rsr[:, ic:ic + 1], rs[:, ic:ic + 1])
                # softmax normalization: scale E' rows in place (DVE 4x)
                nc.vector.tensor_scalar(E_sb[:, ic, :], E_sb[:, ic, :],
                                        rsr[:, ic:ic + 1], None, OP.mult)

            # ---- out = coef^T @ h ----
            out_sb = sbuf.tile([P, EC, D], BF16, tag="osb")
            for jc in range(EC):
                pO = ps_o.tile([P, D], F32, tag="po")
                for ic in range(EC):
                    nc.tensor.matmul(pO[:], E_sb[:, ic, jc * P:(jc + 1) * P],
                                     h_sb[:, ic, :],
                                     start=(ic == 0), stop=(ic == EC - 1))
                nc.scalar.copy(out_sb[:, jc, :], pO[:])
            nc.gpsimd.dma_start(out[n].rearrange("(c p) d -> p c d", p=P), out_sb[:])

        LOOKAHEAD = 2

        def body_all(_iv=None):
            st = {}
            run_prep()
            for n in range(LOOKAHEAD):
                st[n] = load_graph(n)
            for n in range(NG):
                if n + LOOKAHEAD < NG:
                    st[n + LOOKAHEAD] = load_graph(n + LOOKAHEAD)
                compute_graph(n, st.pop(n))

        if reps == 1:
            body_all()
        else:
            with tc.For_i(0, reps, 1) as _iv:
                body_all(_iv)
    return nc


_NC_CACHE = {}
TRACE = False
_LAST = {}


def _get_nc():
    if "nc" not in _NC_CACHE:
        nc = bacc.Bacc("TRN2", target_bir_lowering=False, debug=False)
        build(nc)
        nc.compile()
        _NC_CACHE["nc"] = nc
    return _NC_CACHE["nc"]


def kernel(input_state, adj, entity_mask, query_vec, W_type, a_type,
           qattn_W1, qattn_W2):
    from concourse import bass_utils
    import ml_dtypes
    bf16 = ml_dtypes.bfloat16
    nc = _get_nc()
    x = np.ascontiguousarray(input_state).astype(bf16)
    adjf = np.ascontiguousarray(adj).astype(bf16)
    qvf = np.ascontiguousarray(query_vec, dtype=np.float32)
    Wt = np.ascontiguousarray(W_type).astype(bf16)
    at = np.ascontiguousarray(a_type, dtype=np.float32)
    W1 = np.ascontiguousarray(qattn_W1).astype(bf16)
    W2q = np.ascontiguousarray(qattn_W2).astype(bf16)

    in_maps = []
    for c in range(N_CORES):
        sl = slice(c * NG, (c + 1) * NG)
        in_maps.append({
            "x": x[sl], "adjf": adjf[sl], "qv": qvf[sl],
            "Wt": Wt, "at": at, "W1": W1, "W2q": W2q,
        })
    res = bass_utils.run_bass_kernel_spmd(nc, in_maps, core_ids=list(range(N_CORES)),
                                          trace=TRACE, stitch_traces=TRACE)
    _LAST["exec_ns"] = res.exec_time_ns
    _LAST["mean_ns"] = res.mean_exec_time_ns
    _LAST["trace"] = res.instructions_and_trace
    out = np.concatenate([r["out"] for r in res.results], axis=0)
    return out.astype(np.float32)


# revision 37
# speedup vs baseline: 1.0762x; 1.0122x over previous
"""GAT self-attention Trainium2 kernel (v2, bf16 data path).

Full inputs -> shard graphs over 8 NeuronCores -> full output.

Math (per graph n, reference reformulated):
  g_i = sigmoid(relu(q @ W1_i) @ W2_i)            [2d]
  u_i^L = W_i @ (g_i[:d] * a_i[:d])               [k]   (left projector)
  u_i^R = W_i @ (g_i[d:] * a_i[d:])               [k]   (right projector)
  l_i = X @ u_i^L ; r_i = X @ u_i^R               [E]
  S[i,j] = lrelu(l_t[i] + r_t[j]), t = adj[i,j]
  E' = exp(S) * (adj > 0); rs = rowsum(E')
  h = X @ W_2 ; hs = h / rs[:, None]
  out = E'^T @ hs          (== softmax(scores)^T @ (X @ W_2))

Key implementation points:
  - everything bf16 except the score rank-2 matmuls (f32r) and PSUM.
  - adj shipped as bf16 so type masks are DVE tensor_scalar is_equal in
    4x mode; no gpsimd is_equal, no int32 adj DMA.
  - one DMA per tensor (HWDGE slot costs ~630ns per DMA instruction).
  - scores: per type a single rank-2 matmul from a persistent 12-row
    stack [1s, l_t, r_t, 1s] built by one stt pass from the LR matmul.
  - type select via 2 copy_predicated; adj==0 handled by multiplying
    exp by (adj>0) in the same stt pass that row-sums E'.
  - softmax normalization folded into h's PSUM->SBUF copy (scale by
    1/rs), so no extra pass over the [E,E] matrix.
"""
import numpy as np
from contextlib import ExitStack

import concourse.bass as bass
import concourse.tile as tile
from concourse import mybir, bacc
from concourse.masks import make_identity

F32 = mybir.dt.float32
F32R = mybir.dt.float32r
BF16 = mybir.dt.bfloat16
U8 = mybir.dt.uint8
AF = mybir.ActivationFunctionType
OP = mybir.AluOpType

N_CORES = 8
N, E, K, D = 64, 512, 512, 512   # graphs, entities, in_dim, out_dim
NG = N // N_CORES                # graphs per core
NT = 3                           # edge types
P = 128
EC = E // P                      # 4 partition chunks of E
KC = K // P
DC2 = (2 * D) // P               # 8 chunks of the 2d gate dim


def build(nc, reps=1):
    x = nc.dram_tensor("x", [NG, E, K], BF16, kind="ExternalInput").ap()
    adjf = nc.dram_tensor("adjf", [NG, E, E], BF16, kind="ExternalInput").ap()
    qv = nc.dram_tensor("qv", [NG, K], F32, kind="ExternalInput").ap()
    Wt = nc.dram_tensor("Wt", [NT, K, D], BF16, kind="ExternalInput").ap()
    at = nc.dram_tensor("at", [NT, 2 * D], F32, kind="ExternalInput").ap()
    W1 = nc.dram_tensor("W1", [NT, K, 2 * D], BF16, kind="ExternalInput").ap()
    W2q = nc.dram_tensor("W2q", [NT, 2 * D, 2 * D], BF16, kind="ExternalInput").ap()
    out = nc.dram_tensor("out", [NG, E, D], BF16, kind="ExternalOutput").ap()
    nc._gat_io = (x, adjf, qv, Wt, at, W1, W2q, out)
    _build_once(nc, reps)


def _build_once(nc, reps=1):
    x, adjf, qv, Wt, at, W1, W2q, out = nc._gat_io
    with tile.TileContext(nc) as tc, ExitStack() as ctx:
        # ---------------- persistent tiles ----------------
        pers = ctx.enter_context(tc.tile_pool(name="pers", bufs=1))
        identb = pers.tile([P, P], BF16)
        make_identity(nc, identb[:])
        identf = pers.tile([P, P], F32)
        make_identity(nc, identf[:])
        # U6[k%128, kc, c, n]: c in 0..2 -> left type c, 3..5 -> right
        U6 = pers.tile([P, KC, 2 * NT, NG], BF16)
        Wt2_sb = pers.tile([P, KC, D], BF16)
        nc.scalar.dma_start(Wt2_sb[:], Wt[2].rearrange("(c p) d -> p c d", p=P))
        aT = pers.tile([P, DC2, NT], F32)
        # Score-stack tiles: operand pair for type t at base partition 32*t
        # (compute engines may only touch partition ranges based at 0/32/64/96,
        # so data rows are scattered by DMA, ones rows by legal memset or a
        # one-time DMA).  lhsT_t = stkL[32t:32t+2] = [1s; l_t];
        # rhs_t = stkR[32t:32t+2] = [r_t; 1s].  A/B buffering across graphs.
        ones3 = pers.tile([NT, E], F32)
        nc.vector.memset(ones3[:], 1.0)
        stkL = [pers.tile([66, E], F32, name=f"stkL{i}") for i in range(2)]
        stkR = [pers.tile([66, E], F32, name=f"stkR{i}") for i in range(2)]
        for sb in range(2):
            for t in range(NT):
                nc.vector.memset(stkL[sb][32 * t:32 * t + 1, :], 1.0)
            nc.sync.dma_start(stkR[sb][1:66:32, :], ones3[:])

        # ---------------- PSUM pools (8 banks total) ----------------
        ps_s1 = ctx.enter_context(tc.tile_pool(name="ps_s1", bufs=2, space="PSUM"))
        ps_s23 = ctx.enter_context(tc.tile_pool(name="ps_s23", bufs=1, space="PSUM"))
        ps_h = ctx.enter_context(tc.tile_pool(name="ps_h", bufs=2, space="PSUM"))
        ps_o = ctx.enter_context(tc.tile_pool(name="ps_o", bufs=1, space="PSUM"))
        ps_lr = ctx.enter_context(tc.tile_pool(name="ps_lr", bufs=1, space="PSUM"))

        # ---------------- prep: gates + projector vectors ----------------
        def run_prep():
          with tc.tile_pool(name="prep", bufs=2) as prep, \
               tc.tile_pool(name="prepw", bufs=1) as prepw:
            qv_nat = prep.tile([NG, K], F32, tag="qn")
            nc.scalar.dma_start(qv_nat[:], qv)
            at_nat = prep.tile([NT, 2 * D], F32, tag="an")
            nc.scalar.dma_start(at_nat[:], at)
            W1s = []
            for i in range(NT):
                W1_sb = prepw.tile([P, KC, 2 * D], BF16, name=f"W1_{i}", tag=f"w1_{i}")
                nc.scalar.dma_start(W1_sb[:], W1[i].rearrange("(c p) f -> p c f", p=P))
                W1s.append(W1_sb)
            WTs = []
            for i in range(NT):
                WTi = prepw.tile([P, EC, K], BF16, name=f"WT_{i}", tag=f"wt_{i}")
                nc.sync.dma_start_transpose(WTi[:], Wt[i])
                WTs.append(WTi)
            # prefetch the first two W2q's before any gate compute so the
            # weight stream never stalls behind Act-queue compute
            W2s = {}
            for i in range(2):
                W2_sb = prep.tile([P, DC2, 2 * D], BF16, name=f"W2_{i}", tag="w2")
                nc.scalar.dma_start(
                    W2_sb[:, :, 0:D],
                    W2q[i, :, 0:D].rearrange("(c p) f -> p c f", p=P))
                nc.scalar.dma_start(
                    W2_sb[:, :, D:2 * D],
                    W2q[i, :, D:2 * D].rearrange("(c p) f -> p c f", p=P))
                W2s[i] = W2_sb
            # qT[k%128, kc, n] via PE transposes batched in one PSUM tile
            qT = prep.tile([P, KC, NG], BF16, tag="qT")
            qps = ps_s1.tile([P, E], F32, tag="s1")
            for kc in range(KC):
                nc.tensor.transpose(
                    qps[:, kc * NG:(kc + 1) * NG],
                    qv_nat[:, kc * P:(kc + 1) * P], identf[:NG, :NG])
            nc.vector.tensor_copy(
                qT[:], qps[:, 0:KC * NG].rearrange("p (k n) -> p k n", k=KC))
            # aT[d2%128, oc, t] via PE transposes batched in one PSUM tile
            aps = ps_s1.tile([P, E], F32, tag="s1")
            for oc in range(DC2):
                nc.tensor.transpose(
                    aps[:, oc * NT:(oc + 1) * NT],
                    at_nat[:, oc * P:(oc + 1) * P], identf[:NT, :NT])
            nc.vector.tensor_copy(
                aT[:], aps[:, 0:DC2 * NT].rearrange("p (c t) -> p c t", c=DC2))

            for i in range(NT):
                # rr = relu(W1_i^T q): all 8 out-chunks in one PSUM tile
                prr = ps_s23.tile([P, 2, E], F32, tag="s23")
                prrv = prr[:, 0, 0:DC2 * NG].rearrange("p (c n) -> p c n", c=DC2)
                for oc in range(DC2):
                    for kc in range(KC):
                        nc.tensor.matmul(
                            prrv[:, oc, :], W1s[i][:, kc, oc * P:(oc + 1) * P],
                            qT[:, kc, :], start=(kc == 0), stop=(kc == KC - 1))
                rrT = prep.tile([P, DC2, NG], BF16, tag="rrT")
                nc.scalar.activation(rrT[:], prrv[:], AF.Relu)
                # gv = sigmoid(W2q_i^T rr)
                if i in W2s:
                    W2_sb = W2s[i]
                else:
                    W2_sb = prep.tile([P, DC2, 2 * D], BF16, tag="w2")
                    nc.scalar.dma_start(
                        W2_sb[:, :, 0:D],
                        W2q[i, :, 0:D].rearrange("(c p) f -> p c f", p=P))
                    nc.scalar.dma_start(
                        W2_sb[:, :, D:2 * D],
                        W2q[i, :, D:2 * D].rearrange("(c p) f -> p c f", p=P))
                pgv = ps_s23.tile([P, 2, E], F32, tag="s23")
                pgvv = pgv[:, 0, 0:DC2 * NG].rearrange("p (c n) -> p c n", c=DC2)
                for oc in range(DC2):
                    for dc in range(DC2):
                        nc.tensor.matmul(
                            pgvv[:, oc, :], W2_sb[:, dc, oc * P:(oc + 1) * P],
                            rrT[:, dc, :], start=(dc == 0), stop=(dc == DC2 - 1))
                gvT = prep.tile([P, DC2, NG], BF16, tag="gvT")
                nc.scalar.activation(gvT[:], pgvv[:], AF.Sigmoid)
                # vT = gvT * a_i (broadcast over n)
                vT = prep.tile([P, DC2, NG], BF16, tag="vT")
                nc.vector.tensor_tensor(
                    vT[:], gvT[:], aT[:, :, i:i + 1].broadcast_to((P, DC2, NG)),
                    OP.mult)
                # U columns: left at c=i, right at c=3+i
                for sde in range(2):
                    pu = ps_s1.tile([P, E], F32, tag="s1")
                    puv = pu[:, 0:KC * NG].rearrange("p (k n) -> p k n", k=KC)
                    for kc in range(KC):
                        for dc in range(EC):
                            nc.tensor.matmul(
                                puv[:, kc, :],
                                WTs[i][:, dc, kc * P:(kc + 1) * P],
                                vT[:, sde * EC + dc, :],
                                start=(dc == 0), stop=(dc == EC - 1))
                    nc.vector.tensor_copy(U6[:, :, 3 * sde + i, :], puv[:])

        # ---------------- main per-graph pipeline ----------------
        deep = ctx.enter_context(tc.tile_pool(name="deep", bufs=2))
        p_adj = ctx.enter_context(tc.tile_pool(name="p_adj", bufs=4))
        p_xt = ctx.enter_context(tc.tile_pool(name="p_xt", bufs=4))
        p_hs = ctx.enter_context(tc.tile_pool(name="p_hs", bufs=8))
        sbuf = ctx.enter_context(tc.tile_pool(name="sbuf", bufs=2))
        small = ctx.enter_context(tc.tile_pool(name="small", bufs=2))

        def load_graph(n):
            h_eng = nc.vector if n < 4 else nc.scalar
            adj_sb = p_adj.tile([P, EC, E], BF16, tag="adj")
            nc.gpsimd.dma_start(adj_sb[:], adjf[n].rearrange("(c p) j -> p c j", p=P))
            Xt_sb = p_xt.tile([P, KC, E], BF16, tag="Xt")
            nc.sync.dma_start_transpose(Xt_sb[:], x[n])
            # h = X @ W_2 (unscaled) -- only needs Xt + Wt2, runs during prep
            h_sb = p_hs.tile([P, EC, D], BF16, tag="hs")
            for ic in range(EC):
                pH = ps_h.tile([P, D], F32, tag="ph")
                for kc in range(KC):
                    nc.tensor.matmul(pH[:], Xt_sb[:, kc, ic * P:(ic + 1) * P],
                                     Wt2_sb[:, kc, :],
                                     start=(kc == 0), stop=(kc == KC - 1))
                if n < 6:
                    nc.vector.tensor_copy(h_sb[:, ic, :], pH[:])
                else:
                    nc.scalar.copy(h_sb[:, ic, :], pH[:])
            return dict(adj_sb=adj_sb, Xt_sb=Xt_sb, h_sb=h_sb)

        def compute_graph(n, st):
            adj_sb, Xt_sb, h_sb = st["adj_sb"], st["Xt_sb"], st["h_sb"]
            # ---- masks (DVE 4x on bf16) ----
            mz = sbuf.tile([P, EC, E], BF16, tag="mz")
            nc.vector.tensor_scalar(mz[:], adj_sb[:], 0.5, None, OP.is_gt)
            m2 = sbuf.tile([P, EC, E], U8, tag="m2")
            nc.gpsimd.tensor_scalar(m2[:], adj_sb[:], 2.0, None, OP.is_equal)
            m3 = sbuf.tile([P, EC, E], U8, tag="m3")
            nc.gpsimd.tensor_scalar(m3[:], adj_sb[:], 3.0, None, OP.is_equal)

            # ---- LR rows -> score stacks ----
            pLR = ps_lr.tile([2 * NT, E], F32, tag="lr")
            for kc in range(KC):
                nc.tensor.matmul(pLR[:], U6[:, kc, :, n], Xt_sb[:, kc, :],
                                 start=(kc == 0), stop=(kc == KC - 1))
            sL, sR = stkL[n % 2], stkR[n % 2]
            stg = small.tile([2 * NT, E], F32, tag="stg")
            nc.vector.tensor_copy(stg[:], pLR[:])
            # l_t -> row 32t+1 of stkL; r_t -> row 32t of stkR (DMA scatter)
            nc.scalar.dma_start(sL[1:66:32, :], stg[0:NT, :])
            nc.scalar.dma_start(sR[0:65:32, :], stg[NT:2 * NT, :])

            # ---- per-chunk scores ----
            rs = small.tile([P, EC], F32, tag="rs")
            rsr = small.tile([P, EC], F32, tag="rsr")
            E_sb = deep.tile([P, EC, E], BF16, tag="E")
            for ic in range(EC):
                pv1 = ps_s1.tile([P, E], F32, tag="s1")
                nc.tensor.matmul(
                    pv1[:], sL[0:2, ic * P:(ic + 1) * P].bitcast(F32R),
                    sR[0:2, :].bitcast(F32R), start=True, stop=True)
                pv23 = ps_s23.tile([P, 2, E], F32, tag="s23")
                nc.tensor.matmul(
                    pv23[:, 0, :], sL[32:34, ic * P:(ic + 1) * P].bitcast(F32R),
                    sR[32:34, :].bitcast(F32R), start=True, stop=True)
                nc.tensor.matmul(
                    pv23[:, 1, :], sL[64:66, ic * P:(ic + 1) * P].bitcast(F32R),
                    sR[64:66, :].bitcast(F32R), start=True, stop=True)
                nc.vector.copy_predicated(pv1[:], m2[:, ic, :], pv23[:, 0, :])
                nc.vector.copy_predicated(pv1[:], m3[:, ic, :], pv23[:, 1, :])
                lr_sb = small.tile([P, E], BF16, tag="lrl")
                nc.scalar.activation(lr_sb[:], pv1[:], AF.Prelu, alpha=0.2)
                e1_sb = small.tile([P, E], BF16, tag="e1")
                nc.scalar.activation(e1_sb[:], lr_sb[:], AF.Exp)
                # E' = e1 * (adj>0), rowsum into rs
                nc.vector.scalar_tensor_tensor(
                    E_sb[:, ic, :], e1_sb[:], 1.0, mz[:, ic, :],
                    OP.mult, OP.mult, accum_out=rs[:, ic:ic + 1])
                nc.vector.reciprocal(rsr[:, ic:ic + 1], rs[:, ic:ic + 1])
                # softmax normalization: scale E' rows in place (DVE 4x)
                nc.vector.tensor_scalar(E_sb[:, ic, :], E_sb[:, ic, :],
                                        rsr[:, ic:ic + 1], None, OP.mult)

            # ---- out = coef^T @ h ----
            out_sb = sbuf.tile([P, EC, D], BF16, tag="osb")
            for jc in range(EC):
                pO = ps_o.tile([P, D], F32, tag="po")
                for ic in range(EC):
                    nc.tensor.matmul(pO[:], E_sb[:, ic, jc * P:(jc + 1) * P],
                                     h_sb[:, ic, :],
                                     start=(ic == 0), stop=(ic == EC - 1))
                nc.scalar.copy(out_sb[:, jc, :], pO[:])
            nc.gpsimd.dma_start(out[n].rearrange("(c p) d -> p c d", p=P), out_sb[:])

        LOOKAHEAD = 2

        def body_all(_iv=None):
            st = {}
            run_prep()
            for n in range(LOOKAHEAD):
                st[n] = load_graph(n)
            for n in range(NG):
                if n + LOOKAHEAD < NG:
                    st[n + LOOKAHEAD] = load_graph(n + LOOKAHEAD)
                compute_graph(n, st.pop(n))

        if reps == 1:
            body_all()
        else:
            with tc.For_i(0, reps, 1) as _iv:
                body_all(_iv)
    return nc


_NC_CACHE = {}
TRACE = False
_LAST = {}


def _get_nc():
    if "nc" not in _NC_CACHE:
        nc = bacc.Bacc("TRN2", target_bir_lowering=False, debug=False)
        build(nc)
        nc.compile()
        _NC_CACHE["nc"] = nc
    return _NC_CACHE["nc"]


def kernel(input_state, adj, entity_mask, query_vec, W_type, a_type,
           qattn_W1, qattn_W2):
    from concourse import bass_utils
    import ml_dtypes
    bf16 = ml_dtypes.bfloat16
    nc = _get_nc()
    x = np.ascontiguousarray(input_state).astype(bf16)
    adjf = np.ascontiguousarray(adj).astype(bf16)
    qvf = np.ascontiguousarray(query_vec, dtype=np.float32)
    Wt = np.ascontiguousarray(W_type).astype(bf16)
    at = np.ascontiguousarray(a_type, dtype=np.float32)
    W1 = np.ascontiguousarray(qattn_W1).astype(bf16)
    W2q = np.ascontiguousarray(qattn_W2).astype(bf16)

    in_maps = []
    for c in range(N_CORES):
        sl = slice(c * NG, (c + 1) * NG)
        in_maps.append({
            "x": x[sl], "adjf": adjf[sl], "qv": qvf[sl],
            "Wt": Wt, "at": at, "W1": W1, "W2q": W2q,
        })
    res = bass_utils.run_bass_kernel_spmd(nc, in_maps, core_ids=list(range(N_CORES)),
                                          trace=TRACE, stitch_traces=TRACE)
    _LAST["exec_ns"] = res.exec_time_ns
    _LAST["mean_ns"] = res.mean_exec_time_ns
    _LAST["trace"] = res.instructions_and_trace
    out = np.concatenate([r["out"] for r in res.results], axis=0)
    return out.astype(np.float32)
